# revision 1
# baseline (speedup 1.0000x reference)
"""BERT self-attention (B=2, S=2048, H=1024, 16 heads) on 8 TRN2 NeuronCores.

Sharding: tensor-parallel over heads — 2 heads per core. Each core computes
Q/K/V projections for its head slice (contraction over the full hidden dim),
then attention for its (batch, head) pairs, producing the context transposed
[2*64, B*S]. The host concatenates the 8 per-core slices into [B, S, H].

Device-side layout choices:
  - X is fed pre-transposed ([H, B*S]) so projections run with hidden on the
    partition (contraction) axis; Q^T and K^T come out in [d, token] layout,
    which is exactly what the scores matmul needs.
  - Scores are computed transposed (S^T = K Q^T) per 128-wide k-chunk, two
    heads packed into the PE array concurrently via row tiling (contraction
    is only d=64).
  - exp() runs on the scalar engine straight out of PSUM with the additive
    mask folded into the activation bias and 1/sqrt(d) into its scale.
  - The softmax denominator rides along the PV matmul as a 65th column of
    ones in the V operand; it is broadcast across partitions with a K=1
    ones-matmul, inverted on DVE, and multiplied into the context.
  - Matmul operands are fp16 (PE streams 2 bytes/cycle/partition, so fp32
    runs at half rate); accumulation stays fp32 in PSUM.
"""

import sys
import types

sys.path.insert(0, "/opt/trn_rl_repo")

import numpy as np

# NTFF profiling hook (missing from this image's antenv): only needed when
# tracing; install if available, degrade silently otherwise.
try:
    import antenv.axon_hooks  # noqa: F401
except ImportError:
    try:
        from trn_agent_boot.trn_boot import _ntff_profile_via_ctypes

        _m = types.ModuleType("antenv.axon_hooks")
        _hook = _ntff_profile_via_ctypes("/opt/axon/libaxon_pjrt.so")
        _m.get_axon_ntff_profile_hook = lambda: _hook
        _m.set_axon_ntff_profile_hook = lambda h: None
        sys.modules["antenv.axon_hooks"] = _m
    except Exception:
        pass

import concourse.tile as tile
from concourse import bacc, mybir
from concourse.tile_rust import add_dep_helper
from concourse.bass_utils import run_bass_kernel_spmd

F32 = mybir.dt.float32
F16 = mybir.dt.float16
EXP = mybir.ActivationFunctionType.Exp

B, S, H, NHEADS, D = 2, 2048, 1024, 16, 64
T = B * S                # 4096 tokens
DPC = 128                # output dims per core (2 heads x 64)
NCORES = 8
NKC = S // 128           # 16 k-chunks per batch
NQB = S // 512           # 4 q-blocks of 512 per batch
NTB = T // 512           # 8 token blocks of 512
NCI = H // 128           # 8 hidden (contraction) chunks

last_exec_time_ns = None
last_results = None

_cache = {}


def _build():
    nc = bacc.Bacc(
        "TRN2", target_bir_lowering=False, debug=False, enable_asserts=False
    )
    xt = nc.declare_dram_parameter("xt", [H, T], F16, isOutput=False)
    wq = nc.declare_dram_parameter("wq", [H, DPC], F16, isOutput=False)
    wk = nc.declare_dram_parameter("wk", [H, DPC], F16, isOutput=False)
    wv = nc.declare_dram_parameter("wv", [H, DPC], F16, isOutput=False)
    bq = nc.declare_dram_parameter("bq", [DPC, 1], F32, isOutput=False)
    bk = nc.declare_dram_parameter("bk", [DPC, 1], F32, isOutput=False)
    bvb = nc.declare_dram_parameter("bvb", [128, DPC], F32, isOutput=False)
    msk = nc.declare_dram_parameter("msk", [128, B * NKC], F32, isOutput=False)
    ones = nc.declare_dram_parameter("ones", [128, 64], F16, isOutput=False)
    out = nc.declare_dram_parameter("out", [DPC, T], F32, isOutput=True)

    xt_r = xt.rearrange("(c p) t -> p c t", p=128)   # [128, 8, T]
    wq_r = wq.rearrange("(c p) d -> p c d", p=128)   # [128, 8, 128]
    wk_r = wk.rearrange("(c p) d -> p c d", p=128)
    wv_r = wv.rearrange("(c p) d -> p c d", p=128)

    with tile.TileContext(nc) as tc:
        with tc.tile_pool(name="persist", bufs=1) as pp:
            wq_sb = pp.tile([128, NCI, 128], F16, tag="wq")
            wk_sb = pp.tile([128, NCI, 128], F16, tag="wk")
            wv_sb = pp.tile([128, NCI, 128], F16, tag="wv")
            bq_sb = pp.tile([DPC, 1], F32, tag="bq")
            bk_sb = pp.tile([DPC, 1], F32, tag="bk")
            bvb_sb = pp.tile([128, DPC], F32, tag="bvb")
            msk_sb = pp.tile([128, B * NKC], F32, tag="msk")
            ones_sb = pp.tile([128, 64], F16, tag="ones_sb")
            qt_sb = pp.tile([128, T], F16, tag="qt")
            kt_sb = pp.tile([128, T], F16, tag="kt")
            # V in [k, d] layout + a ones column: [128, b, kc, 65] per head
            vx0 = pp.tile([128, B, NKC, 65], F16, tag="vx0")
            vx1 = pp.tile([128, B, NKC, 65], F16, tag="vx1")

            nc.scalar.dma_start(wk_sb[:], wk_r[:])
            nc.scalar.dma_start(wq_sb[:], wq_r[:])
            nc.scalar.dma_start(wv_sb[:], wv_r[:])
            nc.scalar.dma_start(msk_sb[:], msk[:])
            nc.scalar.dma_start(bq_sb[:], bq[:])
            nc.scalar.dma_start(bk_sb[:], bk[:])
            nc.scalar.dma_start(bvb_sb[:], bvb[:])
            nc.scalar.dma_start(ones_sb[:], ones[:])
            for b_i in range(B):
                for kc in range(NKC):
                    nc.vector.memset(vx0[:, b_i, kc, 64:65], 1.0)
                    nc.vector.memset(vx1[:, b_i, kc, 64:65], 1.0)

            # ---- Phase 1: projections ----
            # Batch 0 first, K before Q/V, so batch-0 attention (which needs
            # all of K(b0) but only the first q-block of Q) can start early.
            xtp = tc.alloc_tile_pool(name="xtp", bufs=5)
            if True:
                def dma_xt(tb):
                    xt_t = xtp.tile([128, NCI, 512], F16, tag="xt",
                                    name="xt_t")
                    nc.sync.dma_start(
                        xt_t[:], xt_r[:, :, tb * 512:(tb + 1) * 512]
                    )
                    return xt_t

                b0_tiles = []
                for tb in range(4):
                    b0_tiles.append(dma_xt(tb))

            # ---- Phase 2: attention (batch-1 projections woven in) ----
            with tc.tile_pool(name="stp", bufs=2, space="PSUM") as stp, \
                 tc.tile_pool(name="ctxp", bufs=4, space="PSUM") as ctxp, \
                 tc.tile_pool(name="esp", bufs=6) as esp, \
                 tc.tile_pool(name="smallp", bufs=4) as smallp:
                qt_done = {}
                kt_done = {}
                vx_done = {}

                def proj_q2(tb, xt_t):
                    q_ps = ctxp.tile([128, 512], F32, tag="ctx", name="q_ps2")
                    for ci in range(NCI):
                        nc.tensor.matmul(
                            q_ps[:], wq_sb[:, ci, :], xt_t[:, ci, :],
                            start=(ci == 0), stop=(ci == NCI - 1),
                        )
                    col = tb * 512
                    qt_done[tb] = nc.vector.tensor_scalar_add(
                        qt_sb[:, col:col + 512], q_ps[:], bq_sb[:, 0:1]
                    )

                def proj_k2(tb, xt_t):
                    k_ps = ctxp.tile([128, 512], F32, tag="ctx", name="k_ps2")
                    for ci in range(NCI):
                        nc.tensor.matmul(
                            k_ps[:], wk_sb[:, ci, :], xt_t[:, ci, :],
                            start=(ci == 0), stop=(ci == NCI - 1),
                        )
                    col = tb * 512
                    kt_done[tb] = nc.vector.tensor_scalar_add(
                        kt_sb[:, col:col + 512], k_ps[:], bk_sb[:, 0:1]
                    )

                def proj_v2(tb, xt_t):
                    for tt in range(4):
                        v_ps = ctxp.tile([128, 128], F32, tag="ctx",
                                         name="v_ps2")
                        for ci in range(NCI):
                            nc.tensor.matmul(
                                v_ps[:],
                                xt_t[:, ci, tt * 128:(tt + 1) * 128],
                                wv_sb[:, ci, :],
                                start=(ci == 0), stop=(ci == NCI - 1),
                            )
                        g = tb * 4 + tt
                        b_i, kc = g // NKC, g % NKC
                        i0 = nc.vector.tensor_add(
                            vx0[:, b_i, kc, 0:64], v_ps[:, 0:64],
                            bvb_sb[:, 0:64],
                        )
                        i1 = nc.vector.tensor_add(
                            vx1[:, b_i, kc, 0:64], v_ps[:, 64:128],
                            bvb_sb[:, 64:128],
                        )
                        vx_done[(b_i, kc)] = (i0, i1)
                def normalize(ctx0, ctx1, qcol):
                    cout = smallp.tile([128, 512], F32, tag="cout",
                                       name="cout")
                    for h, ctx in ((0, ctx0), (1, ctx1)):
                        cs = smallp.tile([65, 512], F16, tag=f"cs{h}",
                                         name=f"cs{h}")
                        nc.vector.tensor_copy(cs[:], ctx[:])
                        bcp = ctxp.tile([64, 512], F32, tag="ctx",
                                        name=f"bcp{h}")
                        nc.tensor.matmul(
                            bcp[:], ones_sb[64:65, 0:64], cs[64:65, :],
                            start=True, stop=True, tile_position=(64, 0),
                        )
                        rb = smallp.tile([64, 512], F32, tag=f"rb{h}",
                                         name=f"rb{h}")
                        rsc = smallp.tile([64, 512], F32, tag=f"rsc{h}",
                                          name=f"rsc{h}")
                        nc.vector.reciprocal_approx_accurate(
                            rb[:], bcp[:], rsc[:]
                        )
                        nc.vector.tensor_mul(
                            cout[h * 64:(h + 1) * 64, :],
                            cs[0:64, :], rb[:],
                        )
                    nc.sync.dma_start(out[:, qcol:qcol + 512], cout[:])

                # Work queue of small projection chunks, drained a few
                # matmuls at a time between score chunks so the scalar
                # engine (the bottleneck) is never starved.
                work = []

                def wq_proj(kind, tb, xt_t):
                    # split one projection into 4 chunks of 2 ci-steps
                    state = {}

                    def chunk(ci0, kind=kind, tb=tb, xt_t=xt_t, state=state):
                        w_sb, t_sb, b_sb = (
                            (wq_sb, qt_sb, bq_sb) if kind == "q"
                            else (wk_sb, kt_sb, bk_sb)
                        )
                        if ci0 == 0:
                            state["ps"] = ctxp.tile(
                                [128, 512], F32, tag="ctx", name="pj_ps"
                            )
                        ps_t = state["ps"]
                        for ci in (ci0, ci0 + 1):
                            nc.tensor.matmul(
                                ps_t[:], w_sb[:, ci, :], xt_t[:, ci, :],
                                start=(ci == 0), stop=(ci == NCI - 1),
                            )
                        if ci0 == NCI - 2:
                            col = tb * 512
                            done = (qt_done if kind == "q" else kt_done)
                            done[tb] = nc.vector.tensor_scalar_add(
                                t_sb[:, col:col + 512], ps_t[:], b_sb[:, 0:1]
                            )
                    for ci0 in range(0, NCI, 2):
                        work.append(lambda c=ci0: chunk(c))

                def wq_vproj(tb, xt_t):
                    state = {}

                    def vhalf(tt, half, tb=tb, xt_t=xt_t, state=state):
                        if half == 0:
                            state[tt] = ctxp.tile([128, 128], F32, tag="ctx",
                                                  name="v_ps2")
                        v_ps = state[tt]
                        for ci in range(half * 4, half * 4 + 4):
                            nc.tensor.matmul(
                                v_ps[:],
                                xt_t[:, ci, tt * 128:(tt + 1) * 128],
                                wv_sb[:, ci, :],
                                start=(ci == 0), stop=(ci == NCI - 1),
                            )
                        if half == 1:
                            g = tb * 4 + tt
                            b_i, kc = g // NKC, g % NKC
                            i0 = nc.vector.tensor_add(
                                vx0[:, b_i, kc, 0:64], v_ps[:, 0:64],
                                bvb_sb[:, 0:64],
                            )
                            i1 = nc.vector.tensor_add(
                                vx1[:, b_i, kc, 0:64], v_ps[:, 64:128],
                                bvb_sb[:, 64:128],
                            )
                            vx_done[(b_i, kc)] = (i0, i1)
                    for tt in range(4):
                        work.append(lambda t=tt: vhalf(t, 0))
                        work.append(lambda t=tt: vhalf(t, 1))

                def wq_ps1(tb):
                    # K then V for one b0 token-block, through ps1 psums,
                    # split into small chunks
                    state = {}

                    def kchunk(ci0, tb=tb, state=state):
                        if ci0 == 0:
                            state["k"] = ctxp.tile([128, 512], F32, tag="ctx",
                                                   name="k_ps")
                        k_ps = state["k"]
                        for ci in (ci0, ci0 + 1):
                            nc.tensor.matmul(
                                k_ps[:], wk_sb[:, ci, :],
                                b0_tiles[tb][:, ci, :],
                                start=(ci == 0), stop=(ci == NCI - 1),
                            )
                        if ci0 == NCI - 2:
                            col = tb * 512
                            kt_done[tb] = nc.vector.tensor_scalar_add(
                                kt_sb[:, col:col + 512], k_ps[:],
                                bk_sb[:, 0:1]
                            )
                    for ci0 in range(0, NCI, 2):
                        work.append(lambda c=ci0: kchunk(c))

                    def vchunk(tt, tb=tb):
                        v_ps = ctxp.tile([128, 128], F32, tag="ctx",
                                         name="v_ps")
                        for ci in range(NCI):
                            nc.tensor.matmul(
                                v_ps[:],
                                b0_tiles[tb][:, ci, tt * 128:(tt + 1) * 128],
                                wv_sb[:, ci, :],
                                start=(ci == 0), stop=(ci == NCI - 1),
                            )
                        g = tb * 4 + tt
                        bb, kcc = g // NKC, g % NKC
                        i0 = nc.vector.tensor_add(
                            vx0[:, bb, kcc, 0:64], v_ps[:, 0:64],
                            bvb_sb[:, 0:64],
                        )
                        i1 = nc.vector.tensor_add(
                            vx1[:, bb, kcc, 0:64], v_ps[:, 64:128],
                            bvb_sb[:, 64:128],
                        )
                        vx_done[(bb, kcc)] = (i0, i1)
                    for tt in range(4):
                        work.append(lambda t=tt: vchunk(t))

                def emit_v_tt(tb, tt, emit=None):
                    v_ps = ctxp.tile([128, 128], F32, tag="ctx",
                                     name="v_ps")
                    for ci in range(NCI):
                        nc.tensor.matmul(
                            v_ps[:],
                            b0_tiles[tb][:, ci, tt * 128:(tt + 1) * 128],
                            wv_sb[:, ci, :],
                            start=(ci == 0), stop=(ci == NCI - 1),
                        )
                    g = tb * 4 + tt
                    bb, kcc = g // NKC, g % NKC
                    i0 = nc.vector.tensor_add(
                        vx0[:, bb, kcc, 0:64], v_ps[:, 0:64],
                        bvb_sb[:, 0:64],
                    )
                    i1 = nc.vector.tensor_add(
                        vx1[:, bb, kcc, 0:64], v_ps[:, 64:128],
                        bvb_sb[:, 64:128],
                    )
                    vx_done[(bb, kcc)] = (i0, i1)

                def filler(b_i, qb):
                    if b_i == 0 and qb == 0:
                        for tt in (1, 2, 3):
                            work.append(lambda t=tt: emit_v_tt(0, t))
                        for tb in (1, 2, 3):
                            wq_ps1(tb)
                        for tb in (1, 2, 3):
                            wq_proj("q", tb, b0_tiles[tb])
                    elif b_i == 0 and qb == 1:
                        for tb in range(4, 8):
                            b1_tiles.append(dma_xt(tb))
                        for tb in (4, 5, 6, 7):
                            wq_proj("k", tb, b1_tiles[tb - 4])
                    elif b_i == 0 and qb == 2:
                        for tb in (4, 5, 6, 7):
                            wq_vproj(tb, b1_tiles[tb - 4])
                        wq_proj("q", 4, b1_tiles[0])
                    elif b_i == 0 and qb == 3:
                        for tb in (5, 6, 7):
                            wq_proj("q", tb, b1_tiles[tb - 4])

                # batch-0 head-start projections (ctx-tag psums)
                proj_k2(0, b0_tiles[0])
                proj_q2(0, b0_tiles[0])

                emit_v_tt(0, 0)

                b1_tiles = []
                pending = None
                for b_i in range(B):
                    for qb in range(NQB):
                        filler(b_i, qb)
                        qcol = b_i * S + qb * 512
                        ctx0 = ctxp.tile([65, 512], F32, tag="ctx")
                        ctx1 = ctxp.tile([65, 512], F32, tag="ctx")
                        for kc in range(NKC):
                            for _ in range(2):
                                if work:
                                    work.pop(0)()
                            ktb = b_i * 4 + kc // 4
                            qtb = b_i * 4 + qb
                            while work and not (
                                ktb in kt_done and qtb in qt_done
                                and (b_i, kc) in vx_done
                            ):
                                work.pop(0)()
                            kcol = b_i * S + kc * 128
                            st = stp.tile([128, 1024], F32, tag="st")
                            m0 = nc.tensor.matmul(
                                st[:, 0:512],
                                kt_sb[0:64, kcol:kcol + 128],
                                qt_sb[0:64, qcol:qcol + 512],
                                start=True, stop=True, tile_position=(0, 0),
                            )
                            m1 = nc.tensor.matmul(
                                st[:, 512:1024],
                                kt_sb[64:128, kcol:kcol + 128],
                                qt_sb[64:128, qcol:qcol + 512],
                                start=True, stop=True, tile_position=(64, 0),
                            )
                            for m in (m0, m1):
                                add_dep_helper(m.ins, kt_done[ktb].ins,
                                               True, "kt ready")
                                add_dep_helper(m.ins, qt_done[qtb].ins,
                                               True, "qt ready")
                            est = esp.tile([128, 1024], F16, tag="est")
                            nc.scalar.activation(
                                est[:], st[:], EXP, scale=0.125,
                                bias=msk_sb[:, b_i * NKC + kc: b_i * NKC + kc + 1],
                            )
                            p0 = nc.tensor.matmul(
                                ctx0[:], vx0[:, b_i, kc, :], est[:, 0:512],
                                start=(kc == 0), stop=(kc == NKC - 1),
                            )
                            p1 = nc.tensor.matmul(
                                ctx1[:], vx1[:, b_i, kc, :], est[:, 512:1024],
                                start=(kc == 0), stop=(kc == NKC - 1),
                            )
                            vd = vx_done[(b_i, kc)]
                            add_dep_helper(p0.ins, vd[0].ins, True, "vx0")
                            add_dep_helper(p1.ins, vd[1].ins, True, "vx1")
                        if pending is not None:
                            normalize(*pending)
                        pending = (ctx0, ctx1, qcol)
                if pending is not None:
                    while work:
                        work.pop(0)()
                if pending is not None:
                    normalize(*pending)
            xtp.release()

    nc.compile()
    return nc


def kernel(hidden_states, attention_mask, Wq, bq, Wk, bk, Wv, bv, trace=False):
    global last_exec_time_ns, last_results
    x = np.asarray(hidden_states, dtype=np.float32)
    mask = np.asarray(attention_mask, dtype=np.float32)
    Wq = np.asarray(Wq, dtype=np.float32)
    Wk = np.asarray(Wk, dtype=np.float32)
    Wv = np.asarray(Wv, dtype=np.float32)
    bq = np.asarray(bq, dtype=np.float32)
    bk = np.asarray(bk, dtype=np.float32)
    bv = np.asarray(bv, dtype=np.float32)

    if "nc" not in _cache:
        _cache["nc"] = _build()
    nc = _cache["nc"]

    xt = np.ascontiguousarray(x.reshape(T, H).T).astype(np.float16)  # [H, T]
    # mask columns: [p, b*16+kc] = mask[b, kc*128+p]
    mcols = np.ascontiguousarray(
        mask.reshape(B, NKC, 128).transpose(2, 0, 1).reshape(128, B * NKC)
    )
    in_maps = []
    for c in range(NCORES):
        sl = slice(c * DPC, (c + 1) * DPC)
        in_maps.append({
            "xt": xt,
            "wq": np.ascontiguousarray(Wq[:, sl]).astype(np.float16),
            "wk": np.ascontiguousarray(Wk[:, sl]).astype(np.float16),
            "wv": np.ascontiguousarray(Wv[:, sl]).astype(np.float16),
            "bq": np.ascontiguousarray(bq[sl, None]),
            "bk": np.ascontiguousarray(bk[sl, None]),
            "bvb": np.ascontiguousarray(
                np.broadcast_to(bv[sl][None, :], (128, DPC))
            ),
            "msk": mcols,
            "ones": np.ones((128, 64), np.float16),
        })

    res = run_bass_kernel_spmd(
        nc, in_maps, core_ids=list(range(NCORES)), trace=trace
    )
    last_exec_time_ns = res.exec_time_ns
    last_results = res

    # assemble: per-core out [128, T] -> [B, S, 128]; concat over cores
    parts = [
        res.results[c]["out"].reshape(DPC, B, S).transpose(1, 2, 0)
        for c in range(NCORES)
    ]
    return np.ascontiguousarray(np.concatenate(parts, axis=2))

